# revision 20
# baseline (speedup 1.0000x reference)
"""MenuLoss Trainium2 kernel (v2: fp16 folded ladders + per-batch Grams, host contract).

Math: loss = mean_b[pen_b] + mean_b[((calT_b - calP_b)/700)^2], where cal_b are
amt-weighted sums of a degree-446 Chebyshev series p over ids.  Device computes,
per batch, a Gram matrix M_b = S^T W (contraction over the batch's 7168 elements
via TensorE, PSUM-accumulated over 56 chunks of 128), where:
  stationary S rows (59): [1, T_1..T_29(x) (evens stored +1), onehot_s(rem) x28,
                           tanh(4*idsP)]          -- x = idsT/111-1
  moving   W rows (25): [amtT*T_f(y) f<15 (y=T_30(x), fp32 cascade via
                           T2(T3(T5)) then clamped), amtP*onehot_q(k//28) x8,
                           tanh(4*amtP), 1]
Basis built with fp16 Chebyshev double-step ladders (odd chain on DVE 2x, even
terms via ACT Square with shifted (+1) storage), one-hots via DVE 4x is_equal.
True side: p(x) = sum G_T[f,r] T_f(T_30(x)) U_r(x) (G_T solved on host, fp64);
pred side exact: ids rounded on ACT (RNE), p(28q+s) table G_P.  Penalties ride
along as extra Gram rows; id_range penalty is identically 0 (ids <= 222).
Host unshards the 8 cores' Grams and contracts in fp64.
"""
import functools
import sys
import types
import numpy as np
import numpy.polynomial.chebyshev as Ch

# this container's axon build lacks the NTFF profile hook module; stub it so
# run_bass_kernel_spmd(trace=True) degrades to an untraced run instead of dying
if "antenv.axon_hooks" not in sys.modules:
    _m = types.ModuleType("antenv.axon_hooks")
    _m.get_axon_ntff_profile_hook = lambda: None
    sys.modules["antenv.axon_hooks"] = _m

import concourse.bacc as bacc
import concourse.mybir as mybir
import concourse.tile as tile
from concourse.bass_utils import run_bass_kernel_spmd

AFT = mybir.ActivationFunctionType
ALU = mybir.AluOpType
FP32 = mybir.dt.float32
FP16 = mybir.dt.float16
I16 = mybir.dt.int16
I32 = mybir.dt.int32

N_CORES = 8
B, J = 512, 7 * 16 * 64          # 512 batches, 7168 elements/batch
BC = B // N_CORES                # 64 batches per core
SL = 8                           # batches per slice
NSL = BC // SL                   # 8 slices
CH = 56                          # chunks (columns) per batch
C = SL * CH                      # 448 columns per slice

UT = 30                          # true stationary basis: U_0..U_29
FT = 15                          # true moving basis: W_0..W_14, y = T_30
QP, SP = 8, 28                   # pred split k = 28q + s
N_ABS = 9                        # first N_ABS pred rows are |rem-s| (ACT), host-unfolded
NST = 1 + (UT - 1) + SP + 1      # 59 stationary rows
NMV = FT + QP + 1 + 1            # 25 moving rows
R_OH = 1 + (UT - 1)              # stationary row where onehot_s starts (30)
R_TI = R_OH + SP                 # 58
M_WQ = FT                        # moving row where folded q-onehots start (15)
M_TA = M_WQ + QP                 # 23
M_ONE = M_TA + 1                 # 24

RT2 = float(np.sqrt(2.0))


def _build():
    nc = bacc.Bacc("TRN2", target_bir_lowering=False, debug=False, num_devices=1)
    yp = nc.dram_tensor("yp", [BC, J, 2], FP32, kind="ExternalInput")
    yt = nc.dram_tensor("yt", [BC, J, 2], FP32, kind="ExternalInput")
    gout = nc.dram_tensor("gout", [NST, NSL, SL * NMV], FP32, kind="ExternalOutput")

    bias_np = np.broadcast_to(
        np.array([-RT2, -1.0, -27.0 / 56.0] + [-float(i) for i in range(11)],
                 np.float32), (128, 14)).copy()
    bias_dram = nc.inline_tensor(bias_np, name="bias_const")
    yp_r = yp.ap().rearrange("b (p c) t -> p b c t", p=128)
    yt_r = yt.ap().rearrange("b (p c) t -> p b c t", p=128)

    with tile.TileContext(nc) as tc:
        with (
            tc.tile_pool(name="data", bufs=2) as dpool,
            tc.tile_pool(name="stat", bufs=2) as spool,
            tc.tile_pool(name="mov", bufs=2) as mpool,
            tc.tile_pool(name="scr", bufs=1) as scr,
            tc.tile_pool(name="outp", bufs=2) as opool,
            tc.tile_pool(name="psum", bufs=2, space="PSUM") as ppool,
        ):
            bias_t = scr.tile([128, 14], FP32, tag="bias")
            nc.sync.dma_start(bias_t[:], bias_dram.ap())
            b_rt2 = bias_t[:, 0:1]
            b_m1 = bias_t[:, 1:2]
            b_q = bias_t[:, 2:3]

            osbs = []
            for s in range(NSL):
                bs = slice(s * SL, (s + 1) * SL)
                dT = dpool.tile([128, SL, CH, 2], FP32, tag="dT")
                dP = dpool.tile([128, SL, CH, 2], FP32, tag="dP")
                nc.sync.dma_start(dT[:], yt_r[:, bs, :, :])
                nc.sync.dma_start(dP[:], yp_r[:, bs, :, :])
                flat = lambda ap: ap.rearrange("p b c -> p (b c)")
                idsT, amtT = flat(dT[:, :, :, 0]), flat(dT[:, :, :, 1])
                idsP, amtP = flat(dP[:, :, :, 0]), flat(dP[:, :, :, 1])

                ST = spool.tile([128, NST, C], FP16, tag="ST")
                MV = mpool.tile([128, NMV, C], FP16, tag="MV")
                U = lambda r: ST[:, r, :]
                W = lambda f: MV[:, f, :]

                if s < 2:
                    nc.gpsimd.memset(U(0), 1.0)
                    nc.gpsimd.memset(MV[:, M_ONE, :], 1.0)

                # ---- y = T30(x) fp32 cascade T2(T3(T5(x))) first: it heads the
                # critical path (W-chain) and ping-pongs ACT<->DVE, so its ACT
                # ops must precede the evens in ACT's in-order queue ----
                x32 = scr.tile([128, C], FP32, tag="x32")
                w32 = scr.tile([128, C], FP32, tag="w32")
                a32 = scr.tile([128, C], FP32, tag="a32")
                t5 = scr.tile([128, C], FP32, tag="t5")
                t15 = scr.tile([128, C], FP32, tag="t15")
                nc.scalar.activation(x32[:], idsT, AFT.Copy, scale=1.0 / 111.0,
                                     bias=-1.0)
                nc.scalar.activation(w32[:], idsT, AFT.Square, scale=1.0 / 111.0,
                                     bias=b_m1)               # x^2
                nc.scalar.activation(a32[:], w32[:], AFT.Copy, scale=16.0,
                                     bias=-20.0)
                nc.vector.tensor_tensor(a32[:], a32[:], w32[:], ALU.mult)
                nc.vector.scalar_tensor_tensor(t5[:], a32[:], 5.0, x32[:],
                                               ALU.add, ALU.mult)   # T5
                # seeds / pred index extraction fill ACT while DVE runs
                nc.scalar.activation(U(1), idsT, AFT.Copy, scale=1.0 / 111.0,
                                     bias=-1.0)
                nc.scalar.activation(U(2), idsT, AFT.Square, scale=RT2 / 111.0,
                                     bias=b_rt2)              # 2x^2 = T2+1
                k32 = scr.tile([128, C], I32, tag="k32")
                nc.scalar.activation(k32[:], idsP, AFT.Copy, scale=1.0, bias=0.0)
                nc.scalar.activation(w32[:], t5[:], AFT.Square, scale=RT2,
                                     bias=0.0)                # 2*T5^2
                nc.scalar.activation(a32[:], w32[:], AFT.Copy, scale=2.0,
                                     bias=-3.0)
                nc.vector.tensor_tensor(t15[:], a32[:], t5[:], ALU.mult)  # T15
                ys = scr.tile([128, C], FP16, tag="ys")
                nc.scalar.activation(ys[:], t15[:], AFT.Square, scale=RT2,
                                     bias=0.0)                # 2*T15^2 = T30+1
                ysc = scr.tile([128, C], FP16, tag="ysc")
                nc.vector.tensor_scalar_min(ysc[:], ys[:], 2.0)
                q16 = scr.tile([128, C], I16, tag="q16")
                nc.scalar.activation(q16[:], k32[:], AFT.Copy, scale=1.0 / 28.0,
                                     bias=-27.0 / 56.0)       # floor(k/28) (RNE)

                # ---- true moving side: W_f = amtT * T_f(y) (critical chain) ----
                nc.gpsimd.tensor_copy(W(0), amtT)
                mw = scr.tile([128, C], FP16, tag="mw")
                nc.vector.tensor_tensor(mw[:], W(0), ysc[:], ALU.mult)
                nc.vector.tensor_tensor(W(1), mw[:], W(0), ALU.subtract)
                uy = scr.tile([128, C], FP16, tag="uy")
                nc.vector.tensor_scalar(uy[:], ysc[:], 2.0, 2.0, ALU.mult,
                                        ALU.subtract)          # 2y
                for f in range(2, FT):
                    nc.vector.tensor_tensor(mw[:], uy[:], W(f - 1), ALU.mult)
                    nc.vector.tensor_tensor(W(f), mw[:], W(f - 2), ALU.subtract)

                # ---- true stationary: odd chain on DVE ----
                u = scr.tile([128, C], FP16, tag="u")
                v = scr.tile([128, C], FP16, tag="v")
                nc.vector.tensor_scalar(u[:], U(2), 2.0, 2.0, ALU.mult,
                                        ALU.subtract)          # u = 2*T2
                nc.vector.tensor_scalar(v[:], u[:], 1.0, 1.0, ALU.mult,
                                        ALU.subtract)          # v = 2*T2 - 1
                nc.vector.tensor_tensor(U(3), v[:], U(1), ALU.mult)
                mo = scr.tile([128, C], FP16, tag="mo")
                mo2 = scr.tile([128, C], FP16, tag="mo2")
                for r in range(5, UT, 2):
                    eng, m_ = (nc.vector, mo) if r < 21 else (nc.gpsimd, mo2)
                    eng.tensor_tensor(m_[:], u[:], U(r - 2), ALU.mult)
                    eng.tensor_tensor(U(r), m_[:], U(r - 4), ALU.subtract)

                # ---- pred one-hots / folds ----
                rem = scr.tile([128, C], FP16, tag="rem")
                nc.vector.scalar_tensor_tensor(rem[:], q16[:], -28.0, k32[:],
                                               ALU.mult, ALU.add)  # k - 28q
                aP16 = scr.tile([128, C], FP16, tag="aP16")
                nc.gpsimd.tensor_copy(aP16[:], amtP)
                qm = scr.tile([128, C], FP16, tag="qm")
                for qv in range(QP):
                    nc.vector.tensor_scalar(qm[:], q16[:], float(qv), 1.0,
                                            ALU.is_equal, ALU.mult)
                    nc.gpsimd.tensor_tensor(MV[:, M_WQ + qv, :], qm[:], aP16[:],
                                            ALU.mult)
                for sv in range(SP):
                    if sv < N_ABS:
                        nc.scalar.activation(ST[:, R_OH + sv, :], rem[:],
                                             AFT.Abs, scale=1.0,
                                             bias=bias_t[:, 3 + sv:4 + sv])
                    else:
                        nc.vector.tensor_scalar(ST[:, R_OH + sv, :], rem[:],
                                                float(sv), 1.0, ALU.is_equal,
                                                ALU.mult)

                # ---- evens (ACT, off critical path) + penalties ----
                for r in range(4, UT, 2):
                    h = r // 2
                    if h % 2 == 0:     # input stored shifted (T_h + 1)
                        nc.scalar.activation(U(r), U(h), AFT.Square, scale=RT2,
                                             bias=b_rt2)
                    else:
                        nc.scalar.activation(U(r), U(h), AFT.Square, scale=RT2,
                                             bias=0.0)
                nc.scalar.activation(ST[:, R_TI, :], idsP, AFT.Tanh, scale=4.0,
                                     bias=0.0)
                nc.scalar.activation(MV[:, M_TA, :], amtP, AFT.Tanh, scale=4.0,
                                     bias=0.0)

                # ---- per-batch Grams ----
                ps = ppool.tile([NST, SL * NMV], FP32, tag="gram")
                osb = opool.tile([NST, SL * NMV], FP32, tag=f"osb{s}")
                half = (SL // 2) * NMV
                for j in range(C):
                    bb = j // CH
                    cc = j % CH
                    nc.tensor.matmul(ps[:, bb * NMV:(bb + 1) * NMV],
                                     ST[:, :, j], MV[:, :, j],
                                     start=(cc == 0), stop=(cc == CH - 1))
                    if j == (SL // 2) * CH - 1:
                        # first-half batches done: copy out while PE streams on
                        nc.scalar.copy(osb[:, :half], ps[:, :half])
                nc.scalar.copy(osb[:, half:], ps[:, half:])
                osbs.append(osb)
            for s, osb in enumerate(osbs):
                nc.sync.dma_start(gout.ap()[:, s, :], osb[:])
    nc.compile()
    return nc


@functools.lru_cache(maxsize=2)
def _compiled():
    return _build()


def _fold_G_true(coeffs: np.ndarray) -> np.ndarray:
    """G[f, r]: sum_{f<FT, r<UT} G * T_f(T_30(x)) * (T_r(x) + s_r) == chebval.
    s_r = 1 for even r >= 2 (device stores those shifted).  Exact in fp64."""
    N = 450
    M = np.zeros((N, FT * UT))
    for f in range(FT):
        for r in range(UT):
            col = f * UT + r
            a = 30 * f
            M[a + r, col] += 0.5
            M[abs(a - r), col] += 0.5
            if r >= 2 and r % 2 == 0:
                M[a, col] += 1.0
    c = np.zeros(N)
    c[:len(coeffs)] = coeffs
    g, _, _, _ = np.linalg.lstsq(M, c, rcond=None)
    return g.reshape(FT, UT)


def _fold_G_pred(coeffs: np.ndarray) -> np.ndarray:
    q = np.arange(QP)[:, None]
    sv = np.arange(SP)[None, :]
    k = 28 * q + sv
    GP = Ch.chebval(np.minimum(k, 222) / 111.0 - 1.0, coeffs)
    # device pred-stationary rows i<N_ABS hold |rem-i| instead of onehot(rem):
    # rows = Phi @ onehot-hist; fold Phi^-1 so GP2 @ rows == GP @ onehot-hist
    Phi = np.zeros((SP, SP))
    for i in range(N_ABS):
        Phi[i] = np.abs(np.arange(SP) - i)
    for i in range(N_ABS, SP):
        Phi[i, i] = 1.0
    return GP @ np.linalg.inv(Phi)


def kernel(y_pred: np.ndarray, y: np.ndarray, calories_coeffs: np.ndarray,
           _trace: bool = False):
    coeffs = np.asarray(calories_coeffs, np.float64)
    GT = _fold_G_true(coeffs)
    GP = _fold_G_pred(coeffs)

    ypf = np.ascontiguousarray(y_pred.reshape(B, J, 2), np.float32)
    ytf = np.ascontiguousarray(y.reshape(B, J, 2), np.float32)
    in_maps = []
    for i in range(N_CORES):
        in_maps.append({
            "yp": ypf[i * BC:(i + 1) * BC],
            "yt": ytf[i * BC:(i + 1) * BC],
        })
    nc = _compiled()
    res = run_bass_kernel_spmd(nc, in_maps, list(range(N_CORES)), trace=_trace)

    pens = np.zeros(B)
    diffs = np.zeros(B)
    rmap = [0] + list(range(1, UT))          # stationary row for U_r
    for ci, r in enumerate(res.results):
        g = np.asarray(r["gout"], np.float64)     # [NST, NSL, SL*NMV]
        for s in range(NSL):
            for bi in range(SL):
                Mb = g[:, s, bi * NMV:(bi + 1) * NMV]
                b = ci * BC + s * SL + bi
                calT = np.einsum("fr,rf->", GT, Mb[rmap, :FT])
                calP = np.einsum("qs,sq->", GP, Mb[R_OH:R_OH + SP, M_WQ:M_WQ + QP])
                diffs[b] = (calT - calP) / 700.0
                pens[b] = Mb[R_TI, M_ONE] + Mb[0, M_TA] - 2.0 * Mb[R_TI, M_TA]
    loss = pens.mean() + (diffs ** 2).mean()
    out = np.float32(loss)
    if _trace:
        return out, res
    return out
